# revision 2
# baseline (speedup 1.0000x reference)
"""DeepSeek-V3.1 MoE block (B=2,S=512,H=1024,I=512,E=64,topK=8) on 8 trn2 cores.

Strategy (expert-parallel, sparse dispatch, bf16 streaming):
  - The reference's dense-masked MoE is mathematically top-8 sparse: only the
    top-8 experts per token contribute (mask is 0 elsewhere). We exploit that.
  - Host: router (fp64 numpy, selection margin on this regime is ~4e-6 >>
    rounding noise), top-8 per token, per-expert token gather with capacity
    padding (C = multiple of 32, >= max per-expert load over the slot group).
  - Device, per core c: 8 experts (count-sorted slot assignment balances
    capacities across cores so all 8 cores share one NEFF). Per expert ONE
    packed bf16 DMA [Wg | Wu | Wd | X_e^T] (~3.3MB), then
    Y_e = (silu(X_e Wg) * (X_e Wu)) Wd on the PE in bf16 (PSUM fp32).
    Routing weights are folded into the host-side scatter-add (linear in Wd).
    Shared expert: token-parallel (each core takes 128 of the 1024 tokens).
  - Host: scatter-add per-expert outputs (scaled by routing weight), add shared.

DMA floor per core ~30MB bf16 (vs 59MB fp32 baseline); PE ~53us busy.
"""
import os as _os, sys
try:
    import concourse  # noqa: F401  (env-provided, e.g. axon boot path)
except ImportError:
    for _p in ('/root/.axon_site/_ro/trn_rl_repo', '/opt/trn_rl_repo'):
        if _os.path.isdir(_p) and _p not in sys.path:
            sys.path.append(_p)
import numpy as np
import ml_dtypes

BF16 = ml_dtypes.bfloat16

B, S, H, I, E, TOPK = 2, 512, 1024, 512, 64, 8
T = B * S
NCORES = 8
ELOC = E // NCORES
HC, IC = H // 128, I // 128
TSH = T // NCORES  # shared-expert tokens per core (128)
WCOLS = 3 * HC * I  # packed weight columns per expert (gate|up|down) = 12288

LAST_RESULT = None  # BassKernelResults of the most recent run (for test harness)


def _pmajor(a, nchunk):
    """[nchunk*128, F] -> partition-major [128, nchunk*F] (chunk-row-major)."""
    F = a.shape[1]
    return np.ascontiguousarray(
        a.reshape(nchunk, 128, F).transpose(1, 0, 2).reshape(128, nchunk * F))


def _build(caps):
    import concourse.bacc as bacc
    import concourse.mybir as mybir
    from concourse import tile, masks

    F32 = mybir.dt.float32
    BF = mybir.dt.bfloat16
    SILU = mybir.ActivationFunctionType.Silu

    # blob layout per slot el: [Wg 4096 | Wu 4096 | Wd 4096 | Xg HC*caps[el]]
    # then the shared expert: [Wgs|Wus|Wds 12288 | Xs HC*TSH]
    slotw = [WCOLS + HC * c for c in caps]
    soff = np.concatenate([[0], np.cumsum(slotw)])
    BW = int(soff[-1]) + WCOLS + HC * TSH
    yoff = np.concatenate([[0], np.cumsum(caps)])
    YW = int(yoff[-1])

    nc = bacc.Bacc("TRN2", target_bir_lowering=False, debug=False)

    blob_d = nc.dram_tensor("blob", [128, BW], BF, kind="ExternalInput")
    yg_d = nc.dram_tensor("yg", [YW, H], BF, kind="ExternalOutput")
    ys_d = nc.dram_tensor("ys", [TSH, H], BF, kind="ExternalOutput")

    with tile.TileContext(nc) as tc:
        with (
            tc.tile_pool(name="const", bufs=1) as cpool,
            tc.tile_pool(name="wp", bufs=3) as wpool,
            tc.tile_pool(name="ap", bufs=3) as apool,
            tc.tile_pool(name="ps", bufs=2, space="PSUM") as pspool,
        ):
            ident = cpool.tile([128, 128], BF)
            masks.make_identity(nc, ident[:])

            def ffn_block(bt, rows, r0, C_in, out_ap):
                """One <=128-row block through SwiGLU + down-proj.

                bt: packed tile [128, WCOLS + HC*C_in]; rows at offset r0 of
                the X^T region; out_ap DRAM [rows,H] (bf16).
                """
                g_ps = pspool.tile([128, I], F32, tag="g")
                u_ps = pspool.tile([128, I], F32, tag="u")
                x0 = WCOLS
                for h in range(HC):
                    nc.tensor.matmul(g_ps[:rows],
                                     bt[:, x0 + h * C_in + r0:x0 + h * C_in + r0 + rows],
                                     bt[:, h * I:(h + 1) * I],
                                     start=(h == 0), stop=(h == HC - 1))
                for h in range(HC):
                    nc.tensor.matmul(u_ps[:rows],
                                     bt[:, x0 + h * C_in + r0:x0 + h * C_in + r0 + rows],
                                     bt[:, HC * I + h * I:HC * I + (h + 1) * I],
                                     start=(h == 0), stop=(h == HC - 1))
                a_sb = apool.tile([128, I], BF, tag="a")
                nc.scalar.activation(a_sb[:rows], g_ps[:rows], SILU)
                nc.vector.tensor_mul(a_sb[:rows], a_sb[:rows], u_ps[:rows])
                at_sb = apool.tile([128, IC * 128], BF, tag="at")
                for i in range(IC):
                    t_ps = pspool.tile([128, 128], BF, tag="t")
                    nc.tensor.transpose(t_ps[:, :rows],
                                        a_sb[:rows, i * 128:(i + 1) * 128],
                                        ident[:rows, :rows])
                    nc.vector.tensor_copy(at_sb[:, i * 128:i * 128 + rows], t_ps[:, :rows])
                y_sb = apool.tile([128, H], BF, tag="ysb")
                w0 = 2 * HC * I
                for half in range(2):
                    y_ps = pspool.tile([128, 512], F32, tag="y")
                    for i in range(IC):
                        nc.tensor.matmul(y_ps[:rows], at_sb[:, i * 128:i * 128 + rows],
                                         bt[:, w0 + i * H + 512 * half:w0 + i * H + 512 * (half + 1)],
                                         start=(i == 0), stop=(i == IC - 1))
                    nc.vector.tensor_copy(y_sb[:rows, 512 * half:512 * (half + 1)], y_ps[:rows])
                nc.gpsimd.dma_start(out_ap, y_sb[:rows])

            for e in range(ELOC):
                C = caps[e]
                bt = wpool.tile([128, WCOLS + HC * max(max(caps), TSH)], BF, tag="bt")
                nc.sync.dma_start(bt[:, :slotw[e]], blob_d[:, soff[e]:soff[e + 1]])
                for r0 in range(0, C, 128):
                    rows = min(128, C - r0)
                    ffn_block(bt, rows, r0, C,
                              yg_d[yoff[e] + r0:yoff[e] + r0 + rows, :])

            # shared expert on this core's token slice
            bt = wpool.tile([128, WCOLS + HC * max(max(caps), TSH)], BF, tag="bt")
            nc.sync.dma_start(bt[:, :WCOLS + HC * TSH],
                              blob_d[:, soff[-1]:soff[-1] + WCOLS + HC * TSH])
            ffn_block(bt, TSH, 0, TSH, ys_d[:, :])

    nc.compile()
    return nc


def prepare(hidden_states, router_w, shared_gate_w, shared_up_w, shared_down_w,
            expert_gate_k, expert_up_k, expert_down_k):
    """Host-side routing + dispatch. Returns (nc, in_maps, meta)."""
    x = np.ascontiguousarray(np.asarray(hidden_states, dtype=np.float32).reshape(T, H))
    rw = np.asarray(router_w, dtype=np.float32)

    # ---- routing on host (fp64; selection margin >> fp32 noise) ----
    logits = x.astype(np.float64) @ rw.astype(np.float64)
    aff = 1.0 / (1.0 + np.exp(-logits))
    top_idx = np.argpartition(-aff, TOPK - 1, axis=1)[:, :TOPK]        # [T,8]
    top_vals = np.take_along_axis(aff, top_idx, axis=1)
    top_w = top_vals / (top_vals.sum(axis=1, keepdims=True) + 1e-9)    # [T,8]

    flat_e = top_idx.ravel()
    flat_t = np.repeat(np.arange(T), TOPK)
    flat_w = top_w.ravel()
    order = np.argsort(flat_e, kind="stable")
    se, st, sw = flat_e[order], flat_t[order], flat_w[order]
    counts = np.bincount(flat_e, minlength=E)
    offs = np.concatenate([[0], np.cumsum(counts)])

    # count-sorted assignment: slot el gets the el-th group of 8 heaviest
    # experts (one per core) -> light slots get smaller capacities.
    perm = np.argsort(-counts, kind="stable")          # experts by load desc
    slot_expert = perm.reshape(ELOC, NCORES)           # [slot, core] -> expert
    caps = [int(max(32, -(-counts[slot_expert[el]].max() // 32) * 32))
            for el in range(ELOC)]
    slotw = [WCOLS + HC * c for c in caps]
    soff = np.concatenate([[0], np.cumsum(slotw)])
    BW = int(soff[-1]) + WCOLS + HC * TSH
    yoff = np.concatenate([[0], np.cumsum(caps)])

    nc = _build(caps)

    # bf16 casts (host): weights once, X once
    x_bf = x.astype(BF16)
    egk = np.asarray(expert_gate_k).astype(BF16)
    euk = np.asarray(expert_up_k).astype(BF16)
    edk = np.asarray(expert_down_k).astype(BF16)
    sgw = np.asarray(shared_gate_w).astype(BF16)
    suw = np.asarray(shared_up_w).astype(BF16)
    sdw = np.asarray(shared_down_w).astype(BF16)
    wgs = _pmajor(sgw, HC)
    wus = _pmajor(suw, HC)
    wds = _pmajor(sdw, IC)

    in_maps = []
    for c in range(NCORES):
        blob = np.zeros((128, BW), BF16)
        for el in range(ELOC):
            e = int(slot_expert[el, c])
            C = caps[el]
            toks = st[offs[e]:offs[e + 1]]
            n = len(toks)
            o = soff[el]
            blob[:, o:o + HC * I] = _pmajor(egk[e], HC)
            blob[:, o + HC * I:o + 2 * HC * I] = _pmajor(euk[e], HC)
            blob[:, o + 2 * HC * I:o + WCOLS] = _pmajor(edk[e], IC)
            xe = np.zeros((C, H), BF16)
            xe[:n] = x_bf[toks]
            blob[:, o + WCOLS:o + WCOLS + HC * C] = _pmajor(
                np.ascontiguousarray(xe.T), HC)
        o = soff[-1]
        blob[:, o:o + HC * I] = wgs
        blob[:, o + HC * I:o + 2 * HC * I] = wus
        blob[:, o + 2 * HC * I:o + WCOLS] = wds
        blob[:, o + WCOLS:o + WCOLS + HC * TSH] = _pmajor(
            np.ascontiguousarray(x_bf[TSH * c:TSH * (c + 1)].T), HC)
        in_maps.append({"blob": blob})

    return nc, in_maps, (st, sw, offs, slot_expert, yoff)


def assemble(results, meta):
    st, sw, offs, slot_expert, yoff = meta
    out = np.zeros((T, H), np.float32)
    for c in range(NCORES):
        r = results[c]
        out[TSH * c:TSH * (c + 1)] += np.asarray(r["ys"]).astype(np.float32)
        yg = np.asarray(r["yg"])
        for el in range(ELOC):
            e = int(slot_expert[el, c])
            toks = st[offs[e]:offs[e + 1]]
            ws = sw[offs[e]:offs[e + 1]].astype(np.float32)
            out[toks] += ws[:, None] * yg[yoff[el]:yoff[el] + len(toks)].astype(np.float32)
    return out.reshape(B, S, H)


def kernel(**inputs):
    global LAST_RESULT
    import os, time
    from concourse.bass_utils import run_bass_kernel_spmd
    if os.environ.get("BASS_TRACE"):
        try:
            import antenv.axon_hooks  # noqa: F401
        except ImportError:
            # trace requested but the axon NTFF hook module isn't present in
            # this container -- tracing would crash mid-run; disable it.
            os.environ["BASS_NEVER_TRACE"] = "1"
    nc, in_maps, meta = prepare(**inputs)
    last_err = None
    for attempt in range(3):
        try:
            res = run_bass_kernel_spmd(nc, in_maps, core_ids=list(range(NCORES)))
            break
        except Exception as err:  # transient device faults (e.g. NRT exec errors)
            last_err = err
            time.sleep(5 * (attempt + 1))
    else:
        raise last_err
    LAST_RESULT = res
    return assemble(res.results, meta)


# revision 4
# speedup vs baseline: 1.0343x; 1.0343x over previous
"""DeepSeek-V3.1 MoE block (B=2,S=512,H=1024,I=512,E=64,topK=8) on 8 trn2 cores.

Strategy (expert-parallel, sparse dispatch, bf16 streaming):
  - The reference's dense-masked MoE is mathematically top-8 sparse: only the
    top-8 experts per token contribute (mask is 0 elsewhere). We exploit that.
  - Host: router (fp64 numpy, selection margin on this regime is ~4e-6 >>
    rounding noise), top-8 per token, per-expert token gather with capacity
    padding (C = multiple of 32, >= max per-expert load over the slot group).
  - Device, per core c: 8 experts (count-sorted slot assignment balances
    capacities across cores so all 8 cores share one NEFF). Per expert ONE
    packed bf16 DMA [Wg | Wu | Wd | X_e^T] (~3.3MB), then
    Y_e = (silu(X_e Wg) * (X_e Wu)) Wd on the PE in bf16 (PSUM fp32).
    Routing weights are folded into the host-side scatter-add (linear in Wd).
    Shared expert: token-parallel (each core takes 128 of the 1024 tokens).
  - Host: scatter-add per-expert outputs (scaled by routing weight), add shared.

DMA floor per core ~30MB bf16 (vs 59MB fp32 baseline); PE ~53us busy.
"""
import os as _os, sys
try:
    import concourse  # noqa: F401  (env-provided, e.g. axon boot path)
except ImportError:
    for _p in ('/root/.axon_site/_ro/trn_rl_repo', '/opt/trn_rl_repo'):
        if _os.path.isdir(_p) and _p not in sys.path:
            sys.path.append(_p)
import numpy as np
import ml_dtypes

BF16 = ml_dtypes.bfloat16

B, S, H, I, E, TOPK = 2, 512, 1024, 512, 64, 8
T = B * S
NCORES = 8
ELOC = E // NCORES
HC, IC = H // 128, I // 128
TSH = T // NCORES  # shared-expert tokens per core (128)
WCOLS = 3 * HC * I  # packed weight columns per expert (gate|up|down) = 12288

LAST_RESULT = None  # BassKernelResults of the most recent run (for test harness)


def _pmajor(a, nchunk):
    """[nchunk*128, F] -> partition-major [128, nchunk*F] (chunk-row-major)."""
    F = a.shape[1]
    return np.ascontiguousarray(
        a.reshape(nchunk, 128, F).transpose(1, 0, 2).reshape(128, nchunk * F))


def _build(caps):
    import concourse.bacc as bacc
    import concourse.mybir as mybir
    from concourse import tile, masks

    F32 = mybir.dt.float32
    BF = mybir.dt.bfloat16
    SILU = mybir.ActivationFunctionType.Silu

    # Two DMA streams on independent issue paths (SP-HWDGE and Pool-SWDGE):
    #   blobA per slot: [Wg 4096 | Wu 4096]                 (8192 cols)
    #   blobB per slot: [Xg HC*caps[el] | Wd 4096]
    # Output stores + silu go on ACT (scalar) -- no prerequisite loads there,
    # so nothing computes behind a stalled transfer.
    AW = 2 * HC * I  # 8192
    bslotw = [HC * c + IC * H for c in caps]
    boff = np.concatenate([[0], np.cumsum(bslotw)])
    BWB = int(boff[-1]) + HC * TSH + IC * H
    yoff = np.concatenate([[0], np.cumsum(caps)])
    YW = int(yoff[-1])
    maxc = max(max(caps), TSH)

    nc = bacc.Bacc("TRN2", target_bir_lowering=False, debug=False)

    blobA_d = nc.dram_tensor("blobA", [128, (ELOC + 1) * AW], BF, kind="ExternalInput")
    blobB_d = nc.dram_tensor("blobB", [128, BWB], BF, kind="ExternalInput")
    yg_d = nc.dram_tensor("yg", [YW, H], BF, kind="ExternalOutput")
    ys_d = nc.dram_tensor("ys", [TSH, H], BF, kind="ExternalOutput")

    with tile.TileContext(nc) as tc:
        with (
            tc.tile_pool(name="const", bufs=1) as cpool,
            tc.tile_pool(name="wa", bufs=3) as wapool,
            tc.tile_pool(name="wb", bufs=3) as wbpool,
            tc.tile_pool(name="ap", bufs=3) as apool,
            tc.tile_pool(name="ps", bufs=2, space="PSUM") as pspool,
        ):
            ident = cpool.tile([128, 128], BF)
            masks.make_identity(nc, ident[:])

            def ffn_block(btA, btB, rows, r0, C_in, out_ap):
                """One <=128-row block through SwiGLU + down-proj.

                btA [128, 8192] = [Wg|Wu]; btB [128, HC*C_in + 4096] =
                [X^T | Wd]; rows at offset r0 of the X^T region; out_ap DRAM
                [rows,H] (bf16).
                """
                g_ps = pspool.tile([128, I], F32, tag="g")
                u_ps = pspool.tile([128, I], F32, tag="u")
                for h in range(HC):
                    nc.tensor.matmul(g_ps[:rows],
                                     btB[:, h * C_in + r0:h * C_in + r0 + rows],
                                     btA[:, h * I:(h + 1) * I],
                                     start=(h == 0), stop=(h == HC - 1))
                for h in range(HC):
                    nc.tensor.matmul(u_ps[:rows],
                                     btB[:, h * C_in + r0:h * C_in + r0 + rows],
                                     btA[:, HC * I + h * I:HC * I + (h + 1) * I],
                                     start=(h == 0), stop=(h == HC - 1))
                a_sb = apool.tile([128, I], BF, tag="a")
                nc.scalar.activation(a_sb[:rows], g_ps[:rows], SILU)
                nc.vector.tensor_mul(a_sb[:rows], a_sb[:rows], u_ps[:rows])
                at_sb = apool.tile([128, IC * 128], BF, tag="at")
                for i in range(IC):
                    t_ps = pspool.tile([128, 128], BF, tag="t")
                    nc.tensor.transpose(t_ps[:, :rows],
                                        a_sb[:rows, i * 128:(i + 1) * 128],
                                        ident[:rows, :rows])
                    nc.vector.tensor_copy(at_sb[:, i * 128:i * 128 + rows], t_ps[:, :rows])
                y_sb = apool.tile([128, H], BF, tag="ysb")
                w0 = HC * C_in
                for half in range(2):
                    y_ps = pspool.tile([128, 512], F32, tag="y")
                    for i in range(IC):
                        nc.tensor.matmul(y_ps[:rows], at_sb[:, i * 128:i * 128 + rows],
                                         btB[:, w0 + i * H + 512 * half:w0 + i * H + 512 * (half + 1)],
                                         start=(i == 0), stop=(i == IC - 1))
                    nc.vector.tensor_copy(y_sb[:rows, 512 * half:512 * (half + 1)], y_ps[:rows])
                nc.scalar.dma_start(out_ap, y_sb[:rows])

            for e in range(ELOC):
                C = caps[e]
                btA = wapool.tile([128, AW], BF, tag="ba")
                btB = wbpool.tile([128, HC * maxc + IC * H], BF, tag="bb")
                nc.sync.dma_start(btA[:], blobA_d[:, e * AW:(e + 1) * AW])
                nc.gpsimd.dma_start(btB[:, :bslotw[e]], blobB_d[:, boff[e]:boff[e + 1]])
                for r0 in range(0, C, 128):
                    rows = min(128, C - r0)
                    ffn_block(btA, btB, rows, r0, C,
                              yg_d[yoff[e] + r0:yoff[e] + r0 + rows, :])

            # shared expert on this core's token slice
            btA = wapool.tile([128, AW], BF, tag="ba")
            btB = wbpool.tile([128, HC * maxc + IC * H], BF, tag="bb")
            nc.sync.dma_start(btA[:], blobA_d[:, ELOC * AW:(ELOC + 1) * AW])
            nc.gpsimd.dma_start(btB[:, :HC * TSH + IC * H],
                                blobB_d[:, boff[-1]:boff[-1] + HC * TSH + IC * H])
            ffn_block(btA, btB, TSH, 0, TSH, ys_d[:, :])

    nc.compile()
    return nc


def prepare(hidden_states, router_w, shared_gate_w, shared_up_w, shared_down_w,
            expert_gate_k, expert_up_k, expert_down_k):
    """Host-side routing + dispatch. Returns (nc, in_maps, meta)."""
    x = np.ascontiguousarray(np.asarray(hidden_states, dtype=np.float32).reshape(T, H))
    rw = np.asarray(router_w, dtype=np.float32)

    # ---- routing on host (fp64; selection margin >> fp32 noise) ----
    logits = x.astype(np.float64) @ rw.astype(np.float64)
    aff = 1.0 / (1.0 + np.exp(-logits))
    top_idx = np.argpartition(-aff, TOPK - 1, axis=1)[:, :TOPK]        # [T,8]
    top_vals = np.take_along_axis(aff, top_idx, axis=1)
    top_w = top_vals / (top_vals.sum(axis=1, keepdims=True) + 1e-9)    # [T,8]

    flat_e = top_idx.ravel()
    flat_t = np.repeat(np.arange(T), TOPK)
    flat_w = top_w.ravel()
    order = np.argsort(flat_e, kind="stable")
    se, st, sw = flat_e[order], flat_t[order], flat_w[order]
    counts = np.bincount(flat_e, minlength=E)
    offs = np.concatenate([[0], np.cumsum(counts)])

    # count-sorted assignment: slot el gets the el-th group of 8 heaviest
    # experts (one per core) -> light slots get smaller capacities.
    perm = np.argsort(-counts, kind="stable")          # experts by load desc
    slot_expert = perm.reshape(ELOC, NCORES)           # [slot, core] -> expert
    caps = [int(max(32, -(-counts[slot_expert[el]].max() // 32) * 32))
            for el in range(ELOC)]
    AW = 2 * HC * I
    bslotw = [HC * c + IC * H for c in caps]
    boff = np.concatenate([[0], np.cumsum(bslotw)])
    BWB = int(boff[-1]) + HC * TSH + IC * H
    yoff = np.concatenate([[0], np.cumsum(caps)])

    nc = _build(caps)

    # bf16 casts (host): weights once, X once
    x_bf = x.astype(BF16)
    egk = np.asarray(expert_gate_k).astype(BF16)
    euk = np.asarray(expert_up_k).astype(BF16)
    edk = np.asarray(expert_down_k).astype(BF16)
    wgs = _pmajor(np.asarray(shared_gate_w).astype(BF16), HC)
    wus = _pmajor(np.asarray(shared_up_w).astype(BF16), HC)
    wds = _pmajor(np.asarray(shared_down_w).astype(BF16), IC)

    in_maps = []
    for c in range(NCORES):
        blobA = np.zeros((128, (ELOC + 1) * AW), BF16)
        blobB = np.zeros((128, BWB), BF16)
        for el in range(ELOC):
            e = int(slot_expert[el, c])
            C = caps[el]
            toks = st[offs[e]:offs[e + 1]]
            n = len(toks)
            blobA[:, el * AW:el * AW + HC * I] = _pmajor(egk[e], HC)
            blobA[:, el * AW + HC * I:(el + 1) * AW] = _pmajor(euk[e], HC)
            xe = np.zeros((C, H), BF16)
            xe[:n] = x_bf[toks]
            o = boff[el]
            blobB[:, o:o + HC * C] = _pmajor(np.ascontiguousarray(xe.T), HC)
            blobB[:, o + HC * C:o + bslotw[el]] = _pmajor(edk[e], IC)
        blobA[:, ELOC * AW:ELOC * AW + HC * I] = wgs
        blobA[:, ELOC * AW + HC * I:(ELOC + 1) * AW] = wus
        o = boff[-1]
        blobB[:, o:o + HC * TSH] = _pmajor(
            np.ascontiguousarray(x_bf[TSH * c:TSH * (c + 1)].T), HC)
        blobB[:, o + HC * TSH:o + HC * TSH + IC * H] = wds
        in_maps.append({"blobA": blobA, "blobB": blobB})

    return nc, in_maps, (st, sw, offs, slot_expert, yoff)


def assemble(results, meta):
    st, sw, offs, slot_expert, yoff = meta
    out = np.zeros((T, H), np.float32)
    for c in range(NCORES):
        r = results[c]
        out[TSH * c:TSH * (c + 1)] += np.asarray(r["ys"]).astype(np.float32)
        yg = np.asarray(r["yg"])
        for el in range(ELOC):
            e = int(slot_expert[el, c])
            toks = st[offs[e]:offs[e + 1]]
            ws = sw[offs[e]:offs[e + 1]].astype(np.float32)
            out[toks] += ws[:, None] * yg[yoff[el]:yoff[el] + len(toks)].astype(np.float32)
    return out.reshape(B, S, H)


def kernel(**inputs):
    global LAST_RESULT
    import os, time
    from concourse.bass_utils import run_bass_kernel_spmd
    if os.environ.get("BASS_TRACE"):
        try:
            import antenv.axon_hooks  # noqa: F401
        except ImportError:
            # trace requested but the axon NTFF hook module isn't present in
            # this container -- tracing would crash mid-run; disable it.
            os.environ["BASS_NEVER_TRACE"] = "1"
    nc, in_maps, meta = prepare(**inputs)
    last_err = None
    for attempt in range(3):
        try:
            res = run_bass_kernel_spmd(nc, in_maps, core_ids=list(range(NCORES)))
            break
        except Exception as err:  # transient device faults (e.g. NRT exec errors)
            last_err = err
            time.sleep(5 * (attempt + 1))
    else:
        raise last_err
    LAST_RESULT = res
    return assemble(res.results, meta)
